# revision 25
# baseline (speedup 1.0000x reference)
"""Trainium2 Bass kernel for nn_ExaoneAttention (dense transformer attention).

Full-input contract: kernel(**inputs) takes the unsharded inputs and returns
the full [B, S, D] output. Internally shards across 8 NeuronCores:
2-way data parallel over batch x 4-way tensor parallel over kv heads
(2 kv heads = 8 query heads per core). Each core computes a partial
output through its Wo row-slice; the host sums the 4 partials per batch.

All matmuls run in float32r (full PE rate, ~1e-4 relative rounding).
Attention is computed in the "scoresT" orientation (keys on partitions,
queries on the free dim) so probs feed the PV matmul with no transposes.
Softmax row-sums accumulate on GpSimd (tensor adds + a partition-axis
tensor_reduce), diagonal blocks use causally-shortened matmuls with a
single [128,128] triangle mask, and normalization uses a batched
reciprocal plus a rank-1 broadcast matmul.
"""

import contextlib
import ctypes
import os
import sys
import types

import numpy as np

# ---------------------------------------------------------------------------
# Problem constants (hardcoded per contract)
# ---------------------------------------------------------------------------
B, S, D = 2, 2048, 4096
H, HKV, HD = 32, 8, 128
G = H // HKV
THETA = 10000.0

NCORES = 8
BAT_SHARDS = 2
KV_SHARDS = 4
KVH = HKV // KV_SHARDS  # kv heads per core = 2
QH = KVH * G  # q heads per core = 8
FQ = QH * HD  # 1024
FKV = KVH * HD  # 256
DMC = D // 128  # 32 model-dim chunks

QT = 512  # query tile
NQT = S // QT  # 4
SC = 128  # sequence chunk
NSC = S // SC  # 16
DT = 512  # output d tile
NDT = D // DT  # 8

_SCALE = float(HD) ** -0.5


# ---------------------------------------------------------------------------
# Wait-count legalization: this walrus build rejects instructions carrying
# more than a small number of sync waits (fused fp32/fp32r matmul: >1;
# drain: >4). Hoist excess waits onto standalone NoOps on the same engine
# immediately before the offending instruction; AND-semantics are preserved
# by sequential same-engine execution.
# ---------------------------------------------------------------------------
def _legalize_waits(nc):
    import bass_rust
    import concourse.mybir as mybir

    counter = 0
    for f in nc.m.functions:
        for bb in f.blocks:
            il = bb.instructions
            i = 0
            while i < len(il):
                ins = il[i]
                si = ins.sync_info
                if si is None or len(si.on_wait) <= 1:
                    i += 1
                    continue
                waits = list(si.on_wait)
                pos = i
                for w in waits[1:]:
                    counter += 1
                    nop = mybir.InstNoOp(name=f"lgw-{counter}", ins=[], outs=[])
                    nop.engine = ins.engine
                    nop.sync_info = bass_rust.SyncInfo(on_wait=[w], on_update=[])
                    il.insert(pos, nop)
                    pos += 1
                    i += 1
                ins.sync_info = bass_rust.SyncInfo(
                    on_wait=waits[:1], on_update=list(si.on_update)
                )
                i += 1
    return counter


# ---------------------------------------------------------------------------
# Bass kernel builder (per-core program; same program on all 8 cores)
# ---------------------------------------------------------------------------
def _build_nc():
    import concourse.bass as bass
    import concourse.mybir as mybir
    from concourse.masks import make_identity
    from concourse.tile import TileContext
    from concourse.tile_rust import add_dep_helper

    f32 = mybir.dt.float32
    f32r = mybir.dt.float32r
    AF = mybir.ActivationFunctionType
    ALU = mybir.AluOpType
    AX = mybir.AxisListType

    nc = bass.Bass()

    hiT = nc.declare_dram_parameter("hiT", [D, S], f32, isOutput=False)
    wq = nc.declare_dram_parameter("wq", [D, FQ], f32, isOutput=False)
    wk = nc.declare_dram_parameter("wk", [D, FKV], f32, isOutput=False)
    wv = nc.declare_dram_parameter("wv", [D, FKV], f32, isOutput=False)
    wo = nc.declare_dram_parameter("wo", [FQ, D], f32, isOutput=False)
    ccT = nc.declare_dram_parameter("ccT", [HD, S], f32, isOutput=False)
    ssT = nc.declare_dram_parameter("ssT", [HD, S], f32, isOutput=False)
    dmask = nc.declare_dram_parameter("dmask", [SC, SC], f32, isOutput=False)
    out = nc.declare_dram_parameter("out", [S, D], f32, isOutput=True)

    # internal DRAM staging
    qT_d = nc.dram_tensor("qT_d", [QH, HD, S], f32)
    kT_d = nc.dram_tensor("kT_d", [KVH, HD, S], f32)
    v_d = nc.dram_tensor("v_d", [KVH, S, HD], f32)
    ctxT_d = nc.dram_tensor("ctxT_d", [QH, HD, S], f32)

    hiT_r = hiT[:, :].bitcast(f32r).rearrange("(c p) s -> p c s", p=128)
    wq_r = wq[:, :].bitcast(f32r).rearrange("(c p) f -> p c f", p=128)
    wk_r = wk[:, :].bitcast(f32r).rearrange("(c p) f -> p c f", p=128)
    wv_r = wv[:, :].bitcast(f32r).rearrange("(c p) f -> p c f", p=128)
    wo_r = wo[:, :].bitcast(f32r).rearrange("(h p) d -> p h d", p=128)

    def rope(vec, out_sb, psum, cc, ss):
        """out = rope(psum) elementwise; cc/ss are [64, W] half-tables
        (the full 128-row tables are two stacked copies)."""
        t1 = rope_tmp_pool.tile(list(out_sb.shape), f32, name="rope_t1")
        vec.tensor_mul(t1[:64], psum[:64], cc)
        vec.tensor_mul(t1[64:], psum[64:], cc)
        t2 = rope_tmp_pool.tile(list(out_sb.shape), f32, name="rope_t2")
        vec.tensor_mul(t2[:64], psum[64:], ss)
        vec.tensor_mul(t2[64:], psum[:64], ss)
        vec.tensor_sub(out_sb[:64], t1[:64], t2[:64])
        vec.tensor_add(out_sb[64:], t1[64:], t2[64:])

    with TileContext(nc) as tc, contextlib.ExitStack() as top:
        singles = top.enter_context(tc.tile_pool(name="singles", bufs=1))
        kv_pool = top.enter_context(tc.tile_pool(name="kvp", bufs=1))
        kv_tiles = {}

        def load_kv(kv):
            kt_sb = kv_pool.tile([128, S], f32r, name="kt_res")
            v_sb = kv_pool.tile([128, NSC, HD], f32r, name="v_res")
            for st in range(NQT):
                sl = slice(st * QT, (st + 1) * QT)
                nc.sync.dma_start(
                    out=kt_sb[:, sl], in_=kT_d[kv, :, sl].bitcast(f32r)
                )
                nc.sync.dma_start(
                    out=v_sb[:, st * 4 : (st + 1) * 4, :],
                    in_=v_d[kv, sl, :]
                    .bitcast(f32r)
                    .rearrange("(sc p) d -> p sc d", p=128),
                )
            kv_tiles[kv] = (kt_sb, v_sb)

        tri_sb = singles.tile([SC, SC], f32)
        nc.sync.dma_start(out=tri_sb, in_=dmask[:, :])
        ident = singles.tile([128, 128], f32)
        make_identity(nc, ident)
        ones_t = singles.tile([128, 128], f32)
        nc.vector.memset(ones_t, 1.0)
        ones_kk = singles.tile([128, 128], f32r)
        nc.vector.tensor_copy(ones_kk, ones_t)

        with tc.tile_pool(name="ccss", bufs=1) as ccss_pool, contextlib.ExitStack() as abst:
            rope_tmp_pool = abst.enter_context(tc.tile_pool(name="ropetmp", bufs=2))
            cc_sb = ccss_pool.tile([HD // 2, S], f32)
            ss_sb = ccss_pool.tile([HD // 2, S], f32)

            # wq low half prefetches during phase A
            wq_pool = abst.enter_context(tc.tile_pool(name="wqp", bufs=1))
            wq_lo = wq_pool.tile([128, DMC // 2, FQ], f32r)
            hi_pool = abst.enter_context(tc.tile_pool(name="hiAB", bufs=4))

            # ---------------- Phase A: K/V projection (+rope K, transpose V) --
            with contextlib.ExitStack() as ph:
                wkv_pool = ph.enter_context(tc.tile_pool(name="wkv", bufs=1))
                stage_pool = ph.enter_context(tc.tile_pool(name="stageA", bufs=2))
                psA = ph.enter_context(tc.tile_pool(name="psA", bufs=6, space="PSUM"))
                psT = ph.enter_context(tc.tile_pool(name="psTr", bufs=2, space="PSUM"))

                wk_sb = wkv_pool.tile([128, DMC, FKV], f32r)
                wv_sb = wkv_pool.tile([128, DMC, FKV], f32r)

                for wc in range(4):
                    cs = slice(wc * (DMC // 4), (wc + 1) * (DMC // 4))
                    nc.sync.dma_start(out=wk_sb[:, cs, :], in_=wk_r[:, cs, :])
                    nc.sync.dma_start(out=wv_sb[:, cs, :], in_=wv_r[:, cs, :])
                nc.sync.dma_start(out=cc_sb, in_=ccT[:64, :])
                nc.sync.dma_start(out=ss_sb, in_=ssT[:64, :])

                CGA = 2  # hi chunk-group per slab
                anchors_a = []
                for st in range(NQT):
                    if st >= 2:
                        for half in range(2):
                            c0 = (st - 2) * (DMC // 4) + half * (DMC // 8)
                            cs = slice(c0, c0 + DMC // 8)
                            nc.sync.dma_start(out=wq_lo[:, cs, :], in_=wq_r[:, cs, :])
                    ssl = slice(st * QT, (st + 1) * QT)
                    banks = [psA.tile([128, QT], f32, name="psA") for _ in range(2 * KVH)]
                    for cg in range(DMC // CGA):
                        slab = hi_pool.tile([128, CGA, QT], f32r, name="hi_slab")
                        nc.sync.dma_start(
                            out=slab, in_=hiT_r[:, cg * CGA : (cg + 1) * CGA, ssl]
                        )
                        for cc in range(CGA):
                            c = cg * CGA + cc
                            for fc in range(2 * KVH):  # k0,k1,v0,v1
                                w_sb = wk_sb if fc < KVH else wv_sb
                                fs = slice((fc % KVH) * 128, (fc % KVH) * 128 + 128)
                                mm = nc.tensor.matmul(
                                    banks[fc],
                                    w_sb[:, c, fs],
                                    slab[:, cc, :],
                                    start=(c == 0),
                                    stop=(c == DMC - 1),
                                )
                                if cg == 0 and cc == 0 and fc == 0:
                                    anchors_a.append(mm)
                    for fc in range(2 * KVH):
                        kv = fc % KVH
                        if fc < KVH:
                            kt_sb = stage_pool.tile([128, QT], f32r, name="kt_st")
                            rope(nc.vector, kt_sb, banks[fc], cc_sb[:, ssl], ss_sb[:, ssl])
                            nc.sync.dma_start(out=kT_d[kv, :, ssl].bitcast(f32r), in_=kt_sb)
                        else:
                            vt_st = stage_pool.tile([128, QT], f32, name="vt_st")
                            nc.scalar.copy(vt_st, banks[fc])
                            for j in range(QT // 128):
                                ptr = psT.tile([128, 128], f32, name="ptr")
                                nc.tensor.transpose(ptr, vt_st[:, j * 128 : (j + 1) * 128], ident)
                                vblk = stage_pool.tile([128, 128], f32, name="vblk")
                                nc.scalar.copy(vblk, ptr)
                                s0 = st * QT + j * 128
                                nc.sync.dma_start(out=v_d[kv, s0 : s0 + 128, :], in_=vblk)

            # ---------------- Phase B: Q projection (+rope) -------------------
            with contextlib.ExitStack() as ph:
                wqh_pool = ph.enter_context(tc.tile_pool(name="wqhp", bufs=1))
                stage_pool = ph.enter_context(tc.tile_pool(name="stageB", bufs=3))
                psB = ph.enter_context(tc.tile_pool(name="psB", bufs=8, space="PSUM"))

                load_kv(0)

                HC = DMC // 2
                wq_hi = wqh_pool.tile([128, HC, FQ], f32r)
                for wc in reversed(range(4)):
                    cs = slice(wc * (HC // 4), (wc + 1) * (HC // 4))
                    nc.sync.dma_start(
                        out=wq_hi[:, cs, :], in_=wq_r[:, HC + cs.start : HC + cs.stop, :]
                    )

                def wq_at(c):
                    return wq_lo[:, c, :] if c < HC else wq_hi[:, c - HC, :]

                CGB = 2  # hi chunk-group per slab
                anchors_b = []
                for qt in range(NQT):
                    ssl = slice(qt * QT, (qt + 1) * QT)
                    banks = [psB.tile([128, QT], f32, name="psB") for _ in range(QH)]
                    idx = 0
                    anchors_b.append(None)
                    for cg in reversed(range(DMC // CGB)):
                        slab = hi_pool.tile([128, CGB, QT], f32r, name="hi_slab")
                        nc.sync.dma_start(
                            out=slab, in_=hiT_r[:, cg * CGB : (cg + 1) * CGB, ssl]
                        )
                        for cc in range(CGB):
                            c = cg * CGB + cc
                            for h in range(QH):
                                mm = nc.tensor.matmul(
                                    banks[h],
                                    wq_at(c)[:, h * 128 : (h + 1) * 128],
                                    slab[:, cc, :],
                                    start=(idx == 0),
                                    stop=(idx == DMC - 1),
                                )
                                if anchors_b[-1] is None:
                                    anchors_b[-1] = mm
                            idx += 1
                    for h in range(QH):
                        qt_sb = stage_pool.tile([128, QT], f32r, name="qt_st")
                        rope(nc.vector, qt_sb, banks[h], cc_sb[:, ssl], ss_sb[:, ssl])
                        nc.sync.dma_start(out=qT_d[h, :, ssl].bitcast(f32r), in_=qt_sb)

        # ---------------- Phase C: attention --------------------------------
        # wo prefetch pool opened before C so its chunked DMAs overlap C.
        with tc.tile_pool(name="wop", bufs=1) as wo_pool:
            wo_sb = wo_pool.tile([128, QH, D], f32r)

            with contextlib.ExitStack() as ph:
                q_pool = ph.enter_context(tc.tile_pool(name="qp", bufs=4))
                pt_pool = ph.enter_context(tc.tile_pool(name="ptp", bufs=5))
                misc_pool = ph.enter_context(tc.tile_pool(name="miscC", bufs=3))
                ps_s = ph.enter_context(tc.tile_pool(name="ps_s", bufs=3, space="PSUM"))
                ps_ctx = ph.enter_context(tc.tile_pool(name="ps_ctx", bufs=2, space="PSUM"))
                ps_sbc = ph.enter_context(tc.tile_pool(name="ps_sbc", bufs=2, space="PSUM"))

                sub = 0
                for kv in range(KVH):
                    if kv not in kv_tiles:
                        load_kv(kv)
                    kt_sb, v_sb = kv_tiles[kv]
                    for qt in range(NQT):
                        ssl = slice(qt * QT, (qt + 1) * QT)
                        nk = G * (qt + 1)
                        for gh in range(G // 2):
                            # pace the wo prefetch: one 2MB chunk every other subiter
                            if sub % 2 == 0:
                                wh = sub // 2
                                nc.sync.dma_start(
                                    out=wo_sb[:, wh, :], in_=wo_r[:, wh, :]
                                )
                            sub += 1

                            hpair = (kv * G + 2 * gh, kv * G + 2 * gh + 1)
                            qts, pctxs, sbcs = [], [], []
                            for hh in hpair:
                                qtile = q_pool.tile([128, QT], f32r, name="qt_at")
                                nc.sync.dma_start(
                                    out=qtile, in_=qT_d[hh, :, ssl].bitcast(f32r)
                                )
                                qts.append(qtile)
                                pctxs.append(ps_ctx.tile([128, QT], f32, name="pctx"))
                                sbcs.append(ps_sbc.tile([64, QT], f32, name="sbc"))

                            def pv_sum(prev):
                                pi, pqoff, pts = prev
                                for z in range(2):
                                    nc.tensor.matmul(
                                        pctxs[z][:, pqoff:],
                                        v_sb[:, pi, :],
                                        pts[z][:, pqoff:],
                                        start=(pi == 0),
                                        stop=(pi == nk - 1),
                                        skip_group_check=True,
                                    )
                                    nc.tensor.matmul(
                                        sbcs[z][:, pqoff:],
                                        ones_kk[:, :64],
                                        pts[z][:, pqoff:],
                                        start=(pi == 0),
                                        stop=(pi == nk - 1),
                                        skip_group_check=True,
                                    )

                            prev = None
                            for i in range(nk):
                                diag = i - G * qt
                                qoff = max(diag, 0) * SC
                                pts = []
                                for z in range(2):
                                    pss = ps_s.tile([128, QT], f32, name="pss")
                                    nc.tensor.matmul(
                                        pss[:, qoff:],
                                        kt_sb[:, i * 128 : (i + 1) * 128],
                                        qts[z][:, qoff:],
                                        start=True,
                                        stop=True,
                                    )
                                    if diag >= 0:
                                        nc.vector.tensor_add(
                                            pss[:, qoff : qoff + SC],
                                            pss[:, qoff : qoff + SC],
                                            tri_sb,
                                        )
                                    pt = pt_pool.tile([128, QT], f32r, name="pt")
                                    nc.scalar.activation(
                                        pt[:, qoff:], pss[:, qoff:], AF.Exp, scale=_SCALE
                                    )
                                    pts.append(pt)
                                if prev is not None:
                                    pv_sum(prev)
                                prev = (i, qoff, pts)
                            pv_sum(prev)

                            # tail: drain banks fast, one packed reciprocal
                            # per group. craw absorbs a 1/64 factor; the
                            # ones-matmul broadcast contributes 64x.
                            pack = misc_pool.tile([128, QT], f32, name="pack")
                            nc.vector.tensor_copy(pack[:64], sbcs[0])
                            nc.vector.tensor_copy(pack[64:], sbcs[1])
                            craws = []
                            for z in range(2):
                                craw = misc_pool.tile([128, QT], f32, name="craw")
                                nc.scalar.mul(craw, pctxs[z], 1.0 / 64.0)
                                craws.append(craw)
                            rbc = misc_pool.tile([128, QT], f32r, name="rbc")
                            with nc.allow_low_precision(reason="f32r recip"):
                                nc.vector.reciprocal(rbc, pack)
                            for z in range(2):
                                pbc = ps_s.tile([128, QT], f32, name="pbc", bufs=1)
                                nc.tensor.matmul(
                                    pbc,
                                    ones_kk[z * 64 : (z + 1) * 64, :],
                                    rbc[z * 64 : (z + 1) * 64, :],
                                    start=True,
                                    stop=True,
                                )
                                ctx_sb = misc_pool.tile([128, QT], f32r, name="ctx_sb")
                                nc.vector.tensor_mul(ctx_sb, craws[z], pbc)
                                nc.sync.dma_start(
                                    out=ctxT_d[hpair[z], :, ssl].bitcast(f32r),
                                    in_=ctx_sb,
                                )

            # ------------- Phase D: output projection ------------------------
            with contextlib.ExitStack() as phd:
                cx_pool = phd.enter_context(tc.tile_pool(name="cxp", bufs=3))
                o_pool = phd.enter_context(tc.tile_pool(name="op", bufs=3))
                ps_o = phd.enter_context(tc.tile_pool(name="ps_o", bufs=3, space="PSUM"))

                for sc in range(NSC):
                    cx_sb = cx_pool.tile([128, QH, 128], f32r, name="cx")
                    nc.sync.dma_start(
                        out=cx_sb,
                        in_=ctxT_d[:, :, sc * 128 : (sc + 1) * 128]
                        .bitcast(f32r)
                        .rearrange("h p s -> p h s"),
                    )
                    for dt in range(NDT):
                        po = ps_o.tile([128, DT], f32, name="po")
                        for h in range(QH):
                            nc.tensor.matmul(
                                po,
                                cx_sb[:, h, :],
                                wo_sb[:, h, dt * DT : (dt + 1) * DT],
                                start=(h == 0),
                                stop=(h == QH - 1),
                            )
                        o_sb = o_pool.tile([128, DT], f32, name="o_sb")
                        nc.scalar.copy(o_sb, po)
                        nc.sync.dma_start(
                            out=out[sc * 128 : (sc + 1) * 128, dt * DT : (dt + 1) * DT],
                            in_=o_sb,
                        )

    _legalize_waits(nc)
    return nc


_NC_CACHE = {}
_last_exec_ns = None


def _get_nc():
    if "nc" not in _NC_CACHE:
        _NC_CACHE["nc"] = _build_nc()
    return _NC_CACHE["nc"]


# ---------------------------------------------------------------------------
# Optional NTFF profiling hook (used by the local test harness via
# KERNEL_TRACE=1; grading path leaves it off)
# ---------------------------------------------------------------------------
def _install_ntff_hook(so_path="/opt/axon/libaxon_pjrt.so"):
    if "antenv.axon_hooks" in sys.modules:
        return
    try:
        lib = ctypes.CDLL(so_path)
    except OSError:
        lib = None
    if lib is None or not hasattr(lib, "axon_start_nrt_profile"):
        hook = None
    else:
        lib.axon_start_nrt_profile.argtypes = [
            ctypes.POINTER(ctypes.c_int64),
            ctypes.c_size_t,
        ]
        lib.axon_start_nrt_profile.restype = ctypes.c_int64
        lib.axon_stop_nrt_profile.argtypes = [ctypes.c_char_p]
        lib.axon_stop_nrt_profile.restype = ctypes.c_int64

        @contextlib.contextmanager
        def hook(output_dir, device_ids):
            import jax

            jax.devices()
            if device_ids:
                ids = (ctypes.c_int64 * len(device_ids))(*device_ids)
                rc = lib.axon_start_nrt_profile(ids, len(device_ids))
            else:
                rc = lib.axon_start_nrt_profile(None, 0)
            if rc != 0:
                raise RuntimeError(f"axon_start_nrt_profile rc={rc}")
            try:
                yield
            finally:
                n = lib.axon_stop_nrt_profile(str(output_dir).encode())
                print(f"ntff profile: {n} file(s) -> {output_dir}", file=sys.stderr)

    mod = types.ModuleType("antenv.axon_hooks")
    mod.get_axon_ntff_profile_hook = lambda: hook
    sys.modules["antenv.axon_hooks"] = mod


# ---------------------------------------------------------------------------
# Host entry point
# ---------------------------------------------------------------------------
def kernel(hidden_states, position_ids, attention_mask, Wq, Wk, Wv, Wo):
    global _last_exec_ns
    from concourse import bass_utils

    hidden_states = np.asarray(hidden_states, dtype=np.float32)
    position_ids = np.asarray(position_ids)
    attention_mask = np.asarray(attention_mask)
    Wq = np.asarray(Wq, dtype=np.float32)
    Wk = np.asarray(Wk, dtype=np.float32)
    Wv = np.asarray(Wv, dtype=np.float32)
    Wo = np.asarray(Wo, dtype=np.float32)

    if not np.all(np.asarray(attention_mask) > 0):
        # Spec guarantees an all-ones mask; fall back to a host reference
        # implementation for the general case rather than mis-computing.
        return _host_reference(
            hidden_states, position_ids, attention_mask, Wq, Wk, Wv, Wo
        )

    # rope tables per batch: cc/ss [HD, S] with halves stacked
    half = HD // 2
    inv_freq = 1.0 / (THETA ** (np.arange(0, half, dtype=np.float32) / half))
    ccs, sss = [], []
    for b in range(B):
        freqs = position_ids[b].astype(np.float32)[:, None] * inv_freq[None, :]
        cosT = np.cos(freqs).T.astype(np.float32)  # [64, S]
        sinT = np.sin(freqs).T.astype(np.float32)
        ccs.append(np.ascontiguousarray(np.concatenate([cosT, cosT], axis=0)))
        sss.append(np.ascontiguousarray(np.concatenate([sinT, sinT], axis=0)))

    # intra-block causal triangle: dmask[kk, qq] = 0 if qq >= kk else -1e30
    kk = np.arange(SC)[:, None]
    qq = np.arange(SC)[None, :]
    dmask = np.ascontiguousarray(np.where(qq >= kk, 0.0, -1.0e30).astype(np.float32))

    hiTs = [np.ascontiguousarray(hidden_states[b].T) for b in range(B)]

    in_maps = []
    for c in range(NCORES):
        b = c // KV_SHARDS
        m = c % KV_SHARDS
        qcols = slice(m * FQ, (m + 1) * FQ)
        kvcols = slice(m * FKV, (m + 1) * FKV)
        in_maps.append(
            {
                "hiT": hiTs[b],
                "wq": np.ascontiguousarray(Wq[:, qcols]),
                "wk": np.ascontiguousarray(Wk[:, kvcols]),
                "wv": np.ascontiguousarray(Wv[:, kvcols]),
                "wo": np.ascontiguousarray(Wo[qcols, :]),
                "ccT": ccs[b],
                "ssT": sss[b],
                "dmask": dmask,
            }
        )

    nc = _get_nc()
    trace = os.environ.get("KERNEL_TRACE", "") == "1"
    if trace:
        _install_ntff_hook()
        bass_utils.upload_artifacts = lambda tmpdir: f"local:{tmpdir}"
    res = bass_utils.run_bass_kernel_spmd(
        nc, in_maps, list(range(NCORES)), trace=trace
    )
    _last_exec_ns = res.exec_time_ns

    out = np.zeros((B, S, D), dtype=np.float32)
    for c in range(NCORES):
        out[c // KV_SHARDS] += res.results[c]["out"]
    return out


def _host_reference(hidden_states, position_ids, attention_mask, Wq, Wk, Wv, Wo):
    """Numpy fallback for inputs outside the spec's guarantees."""
    q = (hidden_states @ Wq).reshape(B, S, H, HD)
    k = (hidden_states @ Wk).reshape(B, S, HKV, HD)
    v = (hidden_states @ Wv).reshape(B, S, HKV, HD)

    half = HD // 2
    inv_freq = 1.0 / (THETA ** (np.arange(0, half, dtype=np.float32) / half))
    freqs = position_ids.astype(np.float32)[..., None] * inv_freq
    cos = np.cos(freqs)[:, :, None, :]
    sin = np.sin(freqs)[:, :, None, :]

    def rope(x):
        x1, x2 = x[..., :half], x[..., half:]
        return np.concatenate([x1 * cos - x2 * sin, x2 * cos + x1 * sin], axis=-1)

    q, k = rope(q), rope(k)
    qg = q.reshape(B, S, HKV, G, HD)
    scores = np.einsum("bqhgd,bkhd->bhgqk", qg, k) * (HD**-0.5)
    causal = np.tril(np.ones((S, S), bool))
    mask = causal[None, None, None] & (attention_mask[:, None, None, None, :] > 0)
    scores = np.where(mask, scores, np.finfo(np.float32).min)
    scores = scores - scores.max(axis=-1, keepdims=True)
    probs = np.exp(scores)
    probs = probs / probs.sum(axis=-1, keepdims=True)
    ctx = np.einsum("bhgqk,bkhd->bqhgd", probs, v).reshape(B, S, H * HD)
    return (ctx @ Wo).astype(np.float32)
